# revision 1
# baseline (speedup 1.0000x reference)
"""Multi-head attention (RoPE, 16 heads, D=64) Bass kernel for 8 Trainium2 cores.

Sharding: (batch x head-group) — core i handles batch i//4, heads 4*(i%4)..4*(i%4)+3.
Each core computes its 4 heads end-to-end (qkv proj -> RoPE -> attention -> partial
output projection); the host sums the 4 partial projections per batch (row-parallel
linear), which avoids any device collective.

Device layout is fully "transposed" so every PE matmul contracts over the partition
dim at full width:
  xT [C, T] per batch, Q^T/K^T as 2-head-stacked [128, T] tiles, V as [s, d] tiles.
  S^T = K Q^T per (head-pair, t-block 512, s-chunk 128) with row-packed K=64 matmul
  pairs (2 heads concurrently in the PE array); exp on ACT (1/sqrt(D) folded into the
  host-side Q weights; no max subtraction needed: scores ~ N(0,1) so exp is safe in
  f32); O^T accumulated in PSUM with col-packed M=64 matmul pairs; softmax
  denominators come FREE from the tensor engine: a second col-packed matmul pair with
  an all-ones [128,64] stationary operand accumulates sum_s(exp) into a parallel PSUM
  tile, replicated across partitions so the reciprocal+multiply are lane-aligned
  (no DVE accumulation chain, no gpsimd partition reduce); bf16 matmul operands
  (4x fp32 PE throughput, fp32 PSUM accumulation).
"""
import sys
import numpy as np

for _p in ("/opt/trn_rl_repo",):
    if _p not in sys.path:
        sys.path.insert(0, _p)

import ml_dtypes
import concourse.bass as bass
import concourse.tile as tile
import concourse.mybir as mybir
import concourse.bass_isa as bass_isa
from concourse import bass_utils, bacc

F32 = mybir.dt.float32
BF16 = mybir.dt.bfloat16
EXP = mybir.ActivationFunctionType.Exp

B, T, C = 2, 2048, 1024
H, D = 16, 64
HG = 4            # heads per core
N_CORES = 8
TB = 512          # attention t-block
SC = 128          # attention s-chunk
THETA = 10000.0


def build_program(reps=1):
    """Build the per-core SPMD program (identical on all 8 cores).

    KPHASES env (timing diagnostics only): "all" (default), "qkv" (projection
    + rope + V only), "noproj" (everything but the output projection).
    """
    import os as _os
    phases = _os.environ.get("KPHASES", "all")
    nc = bacc.Bacc("TRN2", target_bir_lowering=False, debug=False,
                   num_devices=N_CORES)

    xT = nc.dram_tensor("xT", [C, T], BF16, kind="ExternalInput").ap()
    wqkT = nc.dram_tensor("wqkT", [C, 8 * D], BF16, kind="ExternalInput").ap()
    wvT = nc.dram_tensor("wvT", [C, HG * D], BF16, kind="ExternalInput").ap()
    wpT = nc.dram_tensor("wpT", [2 * 128, C], BF16, kind="ExternalInput").ap()
    cosT = nc.dram_tensor("cosT", [128, T], F32, kind="ExternalInput").ap()
    sinT = nc.dram_tensor("sinT", [128, T], F32, kind="ExternalInput").ap()
    y = nc.dram_tensor("y", [T, C], F32, kind="ExternalOutput").ap()

    n_cc = C // 128          # 8 contraction chunks
    n_tb = T // TB           # 4 t-blocks
    n_sc = T // SC           # 16 s-chunks
    n_tc = T // 128          # 16 proj t-chunks

    from contextlib import ExitStack
    with tile.TileContext(nc) as tc, ExitStack() as ctx:
        pool = lambda name, bufs, **kw: ctx.enter_context(
            tc.tile_pool(name=name, bufs=bufs, **kw))
        wqp = pool("wq", 1)
        wvp = pool("wv", 1)
        wpp = pool("wp", 1)
        csp = pool("cs", 1)
        xap = pool("xa", 1)
        qkp = pool("qk", 1)
        vvp = pool("vv", 1)
        onp = pool("on", 1)
        ohp = pool("oh", 1)
        mmp = pool("mm", 2, space="PSUM")
        oap = pool("oa", 2, space="PSUM")
        ptp = pool("pt", 4)
        rbp = pool("rb", 2)
        rop = pool("ro", 3)
        t1p = pool("t1", 3)
        t2p = pool("t2", 3)
        ysp = pool("ys", 2)

        for _rep in range(reps):
            # ---- load weights / tables / full xT
            wqk_sb = []
            for cc in range(n_cc):
                t = wqp.tile([128, 8 * D], BF16, tag=f"wqk{cc}")
                nc.sync.dma_start(t[:], wqkT[cc * 128:(cc + 1) * 128, :])
                wqk_sb.append(t)
            wv_sb = []
            for cc in range(n_cc):
                t = wvp.tile([128, HG * D], BF16, tag=f"wv{cc}")
                nc.sync.dma_start(t[:], wvT[cc * 128:(cc + 1) * 128, :])
                wv_sb.append(t)
            wp_sb = []
            for kc in range(2):
                t = wpp.tile([128, C], BF16, tag=f"wp{kc}")
                nc.sync.dma_start(t[:], wpT[kc * 128:(kc + 1) * 128, :])
                wp_sb.append(t)
            cos_sb = csp.tile([128, T], F32, tag="cos")
            nc.sync.dma_start(cos_sb[:], cosT)
            sin_sb = csp.tile([128, T], F32, tag="sin")
            nc.sync.dma_start(sin_sb[:], sinT)
            # full xT resident: one big DMA, [C,T] -> [128, 8*T] (block cc at cols cc*T)
            xt_all = xap.tile([128, n_cc * T], BF16, tag="xa")
            nc.sync.dma_start(
                xt_all[:].rearrange("p (c t) -> p c t", c=n_cc),
                xT.rearrange("(c p) t -> p c t", p=128),
            )
            # all-ones stationary operand for the denominator matmuls
            ones_sb = onp.tile([128, 64], BF16, tag="ones")
            nc.vector.memset(ones_sb[:], 1.0)

            # persistent result tiles
            qkT = [qkp.tile([128, T], BF16, tag=f"qk{m}", name=f"qkT{m}") for m in range(4)]
            v_sb = [vvp.tile([128, HG * D], BF16, tag=f"v{s}", name=f"vsb{s}") for s in range(n_sc)]
            oht = [ohp.tile([128, T], BF16, tag=f"oh{k}", name=f"oht{k}") for k in range(2)]

            # ---- QKV projection helpers
            def proj_m(m):
                # Q^T or K^T m-chunk (128 rows = 2 stacked heads), t-chunks of 1024
                for th in range(T // 1024):
                    ps = mmp.tile([128, 1024], F32, tag="mm", name="ps")
                    for th2 in range(2):
                        t0 = th * 1024 + th2 * 512
                        for cc in range(n_cc):
                            nc.tensor.matmul(
                                ps[:, th2 * 512:(th2 + 1) * 512],
                                wqk_sb[cc][:, m * 128:(m + 1) * 128],
                                xt_all[:, cc * T + t0: cc * T + t0 + 512],
                                start=(cc == 0), stop=(cc == n_cc - 1),
                            )
                    nc.vector.tensor_copy(qkT[m][:, th * 1024:(th + 1) * 1024], ps[:])

            def rope_m(m):
                # RoPE in place on qkT[m], chunks of 512
                for ch in range(T // 512):
                    sl = slice(ch * 512, ch * 512 + 512)
                    rot = rop.tile([128, 512], BF16, name="rot")
                    for q, src in enumerate((32, 0, 96, 64)):
                        nc.sync.dma_start(
                            rot[q * 32:(q + 1) * 32, :], qkT[m][src:src + 32, sl]
                        )
                    t1 = t1p.tile([128, 512], F32, name="t1")
                    nc.vector.tensor_mul(t1[:], qkT[m][:, sl], cos_sb[:, sl])
                    t2 = t2p.tile([128, 512], F32, name="t2")
                    nc.vector.tensor_mul(t2[:], rot[:], sin_sb[:, sl])
                    nc.vector.tensor_add(qkT[m][:, sl], t1[:], t2[:])

            def proj_v():
                for sc in range(n_sc):
                    ps = mmp.tile([128, HG * D], F32, tag="mm", name="psv")
                    for cc in range(n_cc):
                        nc.tensor.matmul(
                            ps[:], xt_all[:, cc * T + sc * 128: cc * T + sc * 128 + 128],
                            wv_sb[cc][:],
                            start=(cc == 0), stop=(cc == n_cc - 1),
                        )
                    nc.vector.tensor_copy(v_sb[sc][:], ps[:])

            # Order: head-pair 0's Q/K first so attention can start while the rest
            # of the projections still run.
            proj_m(0)
            rope_m(0)
            proj_m(2)
            rope_m(2)
            proj_v()
            proj_m(1)
            rope_m(1)
            proj_m(3)
            rope_m(3)

            # ---- attention (per head-pair, t-block, s-chunk)
            for hp in range(2 if phases in ("noproj", "all") else 0):
                QT, KT = qkT[hp], qkT[2 + hp]
                for tb in range(n_tb):
                    tsl = slice(tb * TB, (tb + 1) * TB)
                    o1 = oap.tile([128, TB], F32, tag="o1", name="o1")
                    o2 = oap.tile([128, TB], F32, tag="o2", name="o2")
                    for sc in range(n_sc):
                        ssl = slice(sc * SC, (sc + 1) * SC)
                        s_ps = mmp.tile([128, 2 * TB], F32, tag="mm", name="sps")
                        nc.tensor.matmul(
                            s_ps[:, 0:TB], KT[0:64, ssl], QT[0:64, tsl],
                            start=True, stop=True,
                        )
                        nc.tensor.matmul(
                            s_ps[:, TB:2 * TB], KT[64:128, ssl], QT[64:128, tsl],
                            start=True, stop=True,
                        )
                        pt = ptp.tile([128, 2 * TB], BF16, tag="pt", name="pt")
                        nc.scalar.activation(pt[:], s_ps[:], EXP)
                        nc.tensor.matmul(
                            o1[0:64, :], v_sb[sc][:, hp * 128:hp * 128 + 64],
                            pt[:, 0:TB], start=(sc == 0), stop=(sc == n_sc - 1),
                            skip_group_check=True,
                        )
                        nc.tensor.matmul(
                            o1[64:128, :],
                            v_sb[sc][:, hp * 128 + 64:hp * 128 + 128],
                            pt[:, TB:2 * TB], start=(sc == 0), stop=(sc == n_sc - 1),
                            skip_group_check=True,
                        )
                        nc.tensor.matmul(
                            o2[0:64, :], ones_sb[:],
                            pt[:, 0:TB], start=(sc == 0), stop=(sc == n_sc - 1),
                            skip_group_check=True,
                        )
                        nc.tensor.matmul(
                            o2[64:128, :], ones_sb[:],
                            pt[:, TB:2 * TB], start=(sc == 0), stop=(sc == n_sc - 1),
                            skip_group_check=True,
                        )
                    # softmax normalize: o2 rows are the per-head denominators,
                    # replicated across partitions and lane-aligned with o1.
                    rb = rbp.tile([128, TB], F32, tag="rb", name="rb")
                    nc.vector.reciprocal(rb[:], o2[:])
                    nc.vector.tensor_mul(oht[hp][:, tsl], o1[:], rb[:])

            # ---- output projection (partial: this core's 256 k-columns)
            for tcc in range(n_tc if phases == "all" else 0):
                ksl = slice(tcc * 128, (tcc + 1) * 128)
                y_ps = mmp.tile([128, C], F32, tag="mm", name="yps")
                for jc in range(2):
                    jsl = slice(jc * 512, (jc + 1) * 512)
                    for kc in range(2):
                        nc.tensor.matmul(
                            y_ps[:, jsl], oht[kc][:, ksl], wp_sb[kc][:, jsl],
                            start=(kc == 0), stop=(kc == 1),
                        )
                y_sb = ysp.tile([128, C], F32, tag="ys", name="ysb")
                nc.vector.tensor_copy(y_sb[:], y_ps[:])
                nc.sync.dma_start(y[tcc * 128:(tcc + 1) * 128, :], y_sb[:])


    nc.compile()
    return nc


def make_core_inputs(x, w_qkv, w_proj):
    """Shard + pre-transpose host inputs for the 8 cores."""
    bf16 = ml_dtypes.bfloat16
    # RoPE tables (match reference.rope_cos_sin in fp32)
    inv_freq = 1.0 / (THETA ** (np.arange(0, D, 2, dtype=np.float32) / D))
    t = np.arange(T, dtype=np.float32)
    freqs = np.outer(t, inv_freq).astype(np.float32)       # [T, D//2]
    emb = np.concatenate([freqs, freqs], axis=-1)          # [T, D]
    cos_t = np.cos(emb).T.astype(np.float32)               # [D, T]
    sin_t = np.sin(emb).T.astype(np.float32)               # [D, T]
    sin_signed = sin_t.copy()
    sin_signed[0:D // 2] *= -1.0                           # rows 0..31 get -sin
    cosT = np.ascontiguousarray(np.concatenate([cos_t, cos_t], axis=0))
    sinT = np.ascontiguousarray(np.concatenate([sin_signed, sin_signed], axis=0))

    scale = float(D) ** -0.5                               # folded into Q weights
    in_maps = []
    for core in range(N_CORES):
        b, g = core // 4, core % 4
        heads = range(HG * g, HG * g + HG)
        xTc = np.ascontiguousarray(x[b].T)                 # [C, T]
        q_rows = np.concatenate([w_qkv[64 * h:64 * h + 64] for h in heads], 0)
        q_rows = q_rows * scale
        k_rows = np.concatenate(
            [w_qkv[C + 64 * h:C + 64 * h + 64] for h in heads], 0)
        wqkT = np.ascontiguousarray(np.concatenate([q_rows, k_rows], 0).T)
        v_rows = np.concatenate(
            [w_qkv[2 * C + 64 * h:2 * C + 64 * h + 64] for h in heads], 0)
        wvT = np.ascontiguousarray(v_rows.T)               # [C, 256]
        wpT = np.ascontiguousarray(
            w_proj[:, 256 * g:256 * g + 256].T)            # [256, C]
        in_maps.append({
            "xT": xTc.astype(bf16),
            "wqkT": wqkT.astype(bf16),
            "wvT": wvT.astype(bf16),
            "wpT": wpT.astype(bf16),
            "cosT": cosT,
            "sinT": sinT,
        })
    return in_maps


def gather_output(results):
    """Sum the 4 head-group partials per batch -> [B, T, C]."""
    y = np.zeros((B, T, C), dtype=np.float32)
    for core in range(N_CORES):
        b = core // 4
        y[b] += np.asarray(results[core]["y"], dtype=np.float32)
    return y


_CACHED_NC = None


def _get_program():
    global _CACHED_NC
    if _CACHED_NC is None:
        _CACHED_NC = build_program()
    return _CACHED_NC


def kernel(x, w_qkv, w_proj):
    x = np.asarray(x, dtype=np.float32)
    w_qkv = np.asarray(w_qkv, dtype=np.float32)
    w_proj = np.asarray(w_proj, dtype=np.float32)
    in_maps = make_core_inputs(x, w_qkv, w_proj)
    nc = _get_program()
    res = bass_utils.run_bass_kernel_spmd(nc, in_maps, core_ids=list(range(N_CORES)))
    return gather_output(res.results)

